# revision 13
# baseline (speedup 1.0000x reference)
"""Self-contained Trainium2 Bass kernel for a 2-layer GCN encoder
(PyG GCNConv x2 with LeakyReLU), distributed over 8 NeuronCores.

kernel(**inputs) takes the full unsharded inputs (X [50000,512] f32,
edge_index [2,800000] int64, W1/b1/W2/b2) and returns the full
[50000,128] f32 output.

Structure (v2):
- nodes sharded round-robin by 128-block across 8 cores; each core owns
  bpc=50 blocks, grouped into S=5 position segments of 10 blocks.
- dense phase g = dinv * (X @ W) computed per segment, AllGathered per
  segment (5 small AGs, Shared-output) so message passing can start as
  soon as segment 0 lands and the remaining AGs overlap compute.
- message passing is pass-major: pass s processes ALL dst blocks' edges
  whose source lies in segment s. The running per-block sum is kept in
  SBUF (bf16) and re-injected into PSUM at the start of the next pass
  via an identity matmul; the pass-0 injection doubles as the GCN
  self-loop term. Edges are stored as one continuous 128-slot chunk
  stream per (core, segment) (cross-core max padded per block, so the
  chunk->block covering structure is SPMD-uniform); chunks straddling a
  block boundary are matmul'd twice with disjoint one-hot masks.
- the leaky-relu + dinv scale sink is a single Prelu activation on the
  (otherwise idle) scalar engine.
- layer 2 reuses the exact same gather index stream / one-hot metadata
  (same graph); its dense phase is interleaved segment-by-segment into
  layer 1's final pass so AG2 overlaps mp1's tail.
"""

import sys
if "/opt/trn_rl_repo" not in sys.path:
    sys.path.insert(0, "/opt/trn_rl_repo")

import math
from dataclasses import dataclass, field

import numpy as np
import ml_dtypes

import concourse.bass as bass
import concourse.tile as tile
from concourse import bacc, mybir
from concourse.bass_utils import run_bass_kernel_spmd

FP32 = mybir.dt.float32
BF16 = mybir.dt.bfloat16
I32 = mybir.dt.int32
I16 = mybir.dt.int16


@dataclass
class Cfg:
    n: int          # real node count
    e: int          # real edge count
    d_in: int
    h1: int
    h2: int
    cores: int = 8
    bpc: int = 50   # 128-node dst blocks per core
    segs: int = 5   # position segments (AllGather granularity)
    neg: float = 0.2
    gbufs: int = 12  # gather pool buffers
    shared_ag: bool = False

    @property
    def npad(self):
        return self.cores * self.bpc * 128

    @property
    def shard(self):
        return self.bpc * 128

    @property
    def bps(self):
        return self.bpc // self.segs     # blocks per (core, segment)

    @property
    def segrows(self):
        return self.cores * self.bps * 128   # rows per seg table


@dataclass
class Meta:
    # per seg: number of 128-slot chunks
    nch: list = field(default_factory=list)
    # per seg: list of dma_gather calls (col0_in_idx_tile, nck)
    calls: list = field(default_factory=list)
    # per seg: covering list [(block_p, chunk_idx, global_dst_col)]
    covs: list = field(default_factory=list)
    ncov: int = 0
    idx_cols: int = 0
    bias1_nz: bool = False
    bias2_nz: bool = False


def preprocess(cfg: Cfg, X, edge_index, W1, b1, W2, b2):
    n, npad = cfg.n, cfg.npad
    C, S, BPC, BPS = cfg.cores, cfg.segs, cfg.bpc, cfg.bps
    nblk = npad // 128

    src = np.asarray(edge_index[0], dtype=np.int64)
    dst = np.asarray(edge_index[1], dtype=np.int64)
    E = src.size

    deg = np.bincount(dst, minlength=npad).astype(np.float32) + 1.0
    dinv = (1.0 / np.sqrt(deg)).astype(np.float32)

    # node -> (core, p, lane); segment s = p // BPS
    ids = np.arange(npad, dtype=np.int64)
    nb = ids >> 7
    lane_n = ids & 127
    core_of = nb % C
    p_of = nb // C
    q_of = p_of % BPS
    segrow = core_of * (BPS * 128) + q_of * 128 + lane_n  # row in seg table
    seg_of = p_of // BPS

    e_seg = seg_of[src]
    e_row = segrow[src]
    e_core = core_of[dst]
    e_p = p_of[dst]
    e_lane = (dst & 127)

    key = (e_core * S + e_seg) * BPC + e_p
    cnt = np.bincount(key, minlength=C * S * BPC).reshape(C, S, BPC)
    maxcnt = cnt.max(axis=0)                      # [S, BPC]

    # chunk layout per seg (shared across cores)
    starts = np.zeros((S, BPC + 1), np.int64)
    np.cumsum(maxcnt, axis=1, out=starts[:, 1:])
    L = starts[:, -1]                              # stream length per seg
    nch = [int(math.ceil(int(L[s]) / 128)) for s in range(S)]
    Lpad = [nch[s] * 128 for s in range(S)]

    meta = Meta(
        nch=nch,
        bias1_nz=bool(np.any(np.asarray(b1) != 0)),
        bias2_nz=bool(np.any(np.asarray(b2) != 0)),
    )

    # covering structure per seg: block-major (p asc, chunk asc)
    col = 0
    for s in range(S):
        covs = []
        ends = starts[s].copy()
        ends[-1] = Lpad[s]                         # tail slots -> last block
        for p in range(BPC):
            c0 = int(starts[s, p]) // 128
            c1 = (int(ends[p + 1]) - 1) // 128
            for c in range(c0, c1 + 1):
                covs.append((p, c, col))
                col += 1
        meta.covs.append(covs)
    meta.ncov = col

    # gather call structure per seg: calls of up to 8 chunks
    colbase = 0
    for s in range(S):
        calls = []
        for off in range(0, nch[s], 8):
            nck = min(8, nch[s] - off)
            calls.append((colbase + off * 8, nck))
        meta.calls.append(calls)
        colbase += Lpad[s] // 16
    meta.idx_cols = colbase

    # per-core edge placement
    order = np.lexsort((e_row, e_p, e_seg, e_core))
    okey = key[order]
    # position within (core,seg,p) group
    gstart = np.zeros(C * S * BPC + 1, np.int64)
    np.cumsum(cnt.reshape(-1), out=gstart[1:])
    pos_in_grp = np.arange(E, dtype=np.int64) - gstart[okey]
    # global slot within the (core, seg) stream
    slot = starts[(okey // BPC) % S, okey % BPC] + pos_in_grp

    o_core = okey // (S * BPC)
    o_seg = (okey // BPC) % S
    o_row = e_row[order]
    o_lane = e_lane[order]

    idx_arr = [np.zeros((C, Lpad[s]), np.int16) for s in range(S)]
    dstl = [np.full((C, Lpad[s]), -1, np.int32) for s in range(S)]
    for s in range(S):
        m = o_seg == s
        idx_arr[s][o_core[m], slot[m]] = o_row[m].astype(np.int16)
        dstl[s][o_core[m], slot[m]] = o_lane[m].astype(np.int32)

    # slot -> owning block map per seg (shared)
    sblk = []
    for s in range(S):
        sb = np.zeros(Lpad[s], np.int64)
        ends = starts[s].copy()
        ends[-1] = Lpad[s]
        for p in range(BPC):
            sb[int(starts[s, p]):int(ends[p + 1])] = p
        sblk.append(sb)

    # replicated tensors
    XT = np.zeros((cfg.d_in, npad), np.float32)
    XT[:, :n] = np.asarray(X, np.float32).T
    XT = XT.astype(ml_dtypes.bfloat16)
    W1b = np.asarray(W1, np.float32).astype(ml_dtypes.bfloat16)
    W2b = np.asarray(W2, np.float32).astype(ml_dtypes.bfloat16)
    iota4 = np.ascontiguousarray(np.broadcast_to(
        np.arange(128, dtype=np.float32)[None, None, :],
        (128, 8, 128))).astype(ml_dtypes.bfloat16)
    ident = np.eye(128, dtype=np.float32).astype(ml_dtypes.bfloat16)

    in_maps = []
    for c in range(C):
        # idx tile: concat per-seg streams, 16-partition wrap, tiled to 128
        flat = np.concatenate([idx_arr[s][c] for s in range(S)])
        assert flat.size == meta.idx_cols * 16
        idx_tile = np.ascontiguousarray(
            np.tile(flat.reshape(-1, 16).T, (8, 1)))       # [128, idx_cols]

        # dst one-hot lane columns, one per covering, in cov order
        cols = np.full((meta.ncov, 128), -1, np.int32)
        for s in range(S):
            dl = dstl[s][c]
            sb = sblk[s]
            for (p, ch, col_i) in meta.covs[s]:
                sl = slice(ch * 128, (ch + 1) * 128)
                cols[col_i] = np.where(sb[sl] == p, dl[sl], -1)
        dst_tile = np.ascontiguousarray(cols.T.astype(np.float32)
                                        ).astype(ml_dtypes.bfloat16)

        node_sel = ((np.arange(BPC)[:, None] * C + c) * 128
                    + np.arange(128)[None, :]).reshape(-1)
        dv = dinv[node_sel].reshape(BPC, 128).T            # [128, BPC]
        m = {
            "xt": np.ascontiguousarray(XT[:, node_sel]),
            "w1": W1b, "w2": W2b,
            "idx": idx_tile,
            "dstloc": dst_tile,
            "dinv": np.ascontiguousarray(dv).astype(np.float32),
            "iota4": iota4,
            "ident": ident,
        }
        in_maps.append(m)
    assert not meta.bias1_nz and not meta.bias2_nz, \
        "nonzero GCN biases not supported by this kernel variant"
    return in_maps, meta


def build(cfg: Cfg, meta: Meta, stop_after: str = 'full'):
    nc = bacc.Bacc("TRN2", target_bir_lowering=False, debug=False,
                   num_devices=cfg.cores, num_swdge_queues=4)
    C, S, BPC, BPS = cfg.cores, cfg.segs, cfg.bpc, cfg.bps
    kin, kh1 = cfg.d_in // 128, cfg.h1 // 128
    segrows = cfg.segrows
    AT = mybir.ActivationFunctionType
    OP = mybir.AluOpType
    aspace = "Shared" if cfg.shared_ag else "Local"

    xt = nc.dram_tensor("xt", [cfg.d_in, cfg.shard], BF16, kind="ExternalInput")
    w1 = nc.dram_tensor("w1", [cfg.d_in, cfg.h1], BF16, kind="ExternalInput")
    w2 = nc.dram_tensor("w2", [cfg.h1, cfg.h2], BF16, kind="ExternalInput")
    idx = nc.dram_tensor("idx", [128, meta.idx_cols], I16, kind="ExternalInput")
    dstloc = nc.dram_tensor("dstloc", [128, meta.ncov], BF16, kind="ExternalInput")
    dinv = nc.dram_tensor("dinv", [128, BPC], FP32, kind="ExternalInput")
    iota_d = nc.dram_tensor("iota4", [128, 8, 128], BF16, kind="ExternalInput")
    ident_d = nc.dram_tensor("ident", [128, 128], BF16, kind="ExternalInput")
    out = nc.dram_tensor("out", [cfg.shard, cfg.h2], FP32, kind="ExternalOutput")

    rg = [list(range(C))]
    stop = stop_after

    with tile.TileContext(nc) as tc:
        with (
            tc.tile_pool(name="constp", bufs=1) as constp,
            tc.tile_pool(name="persist", bufs=1) as persist,
            tc.tile_pool(name="dram", bufs=1, space="DRAM") as dram,
            tc.tile_pool(name="ohp", bufs=8) as ohp,
            tc.tile_pool(name="sp", bufs=6) as sp,
            tc.tile_pool(name="pp", bufs=6, space="PSUM") as pp,
        ):
            g1s = [dram.tile([BPS * 128, cfg.h1], BF16, name=f"g1s{s}")
                   for s in range(S)]
            g1f = [dram.tile([segrows, cfg.h1], BF16, name=f"g1f{s}",
                             addr_space=aspace) for s in range(S)]
            z1d = [dram.tile([BPS * 128, cfg.h1], BF16, name=f"z1d{s}")
                   for s in range(S)]
            g2s = [dram.tile([BPS * 128, cfg.h2], BF16, name=f"g2s{s}")
                   for s in range(S)]
            g2f = [dram.tile([segrows, cfg.h2], BF16, name=f"g2f{s}",
                             addr_space=aspace) for s in range(S)]

            # ---- constants ----
            w1sb = constp.tile([128, kin, cfg.h1], BF16)
            for k in range(kin):
                nc.sync.dma_start(w1sb[:, k, :], w1[k * 128:(k + 1) * 128, :])
            w2sb = constp.tile([128, kh1, cfg.h2], BF16)
            for k in range(kh1):
                nc.sync.dma_start(w2sb[:, k, :], w2[k * 128:(k + 1) * 128, :])
            idxsb = constp.tile([128, meta.idx_cols], I16)
            nc.sync.dma_start(idxsb[:], idx[:])
            dstsb = constp.tile([128, meta.ncov], BF16)
            nc.sync.dma_start(dstsb[:], dstloc[:])
            dvsb = constp.tile([128, BPC], FP32)
            nc.sync.dma_start(dvsb[:], dinv[:])
            iotasb = constp.tile([128, 8, 128], BF16)
            nc.sync.dma_start(iotasb[:], iota_d[:])
            identsb = constp.tile([128, 128], BF16)
            nc.sync.dma_start(identsb[:], ident_d[:])

            # persistent SBUF tensors
            g1own = persist.tile([128, BPC, cfg.h1], BF16)
            acc1 = persist.tile([128, BPC, cfg.h1], BF16)
            g2own = persist.tile([128, BPC, cfg.h2], BF16)
            acc2 = persist.tile([128, BPC, cfg.h2], BF16)

            # ---- dense layer 1, per segment, AG per segment ----
            with tc.tile_pool(name="xtp", bufs=1) as xtp:
                xts = xtp.tile([128, kin, cfg.shard], BF16)
                for s in range(S):
                    c0, c1 = s * BPS * 128, (s + 1) * BPS * 128
                    for k in range(kin):
                        nc.sync.dma_start(xts[:, k, c0:c1],
                                          xt[k * 128:(k + 1) * 128, c0:c1])
                    for q in range(BPS):
                        p = s * BPS + q
                        ps = pp.tile([128, 256], FP32, tag="ps")
                        for k in range(kin):
                            nc.tensor.matmul(
                                ps[:], xts[:, k, p * 128:(p + 1) * 128],
                                w1sb[:, k, :],
                                start=(k == 0), stop=(k == kin - 1))
                        nc.scalar.mul(g1own[:, p, :], ps[:], dvsb[:, p:p + 1])
                        nc.sync.dma_start(g1s[s][q * 128:(q + 1) * 128, :],
                                          g1own[:, p, :])
                    if stop != "p1":
                        nc.gpsimd.collective_compute(
                            "AllGather", OP.bypass, replica_groups=rg,
                            ins=[g1s[s].opt()], outs=[g1f[s].opt()])

            if stop in ("p1", "ag1"):
                nc.compile()
                return nc

            qctr = [0]

            def make_gatherer(s, gf, h, gp):
                """Returns (ensure(upto_chunk), tiles): emits dma_gather
                calls lazily so collective triggers queued on gpsimd are not
                stuck behind a whole pass of gather dispatches."""
                tiles = []

                def ensure(upto_chunk):
                    while len(tiles) * 8 <= upto_chunk:
                        (col0, nck) = meta.calls[s][len(tiles)]
                        g = gp.tile([128, 8, h], BF16, tag=f"g{h}", name="g")
                        nc.gpsimd.dma_gather(
                            g[:, 0:nck, :], gf[s][:, :],
                            idxsb[:, col0:col0 + nck * 8],
                            nck * 128, nck * 128, h,
                            queue_num=qctr[0] % 4)
                        qctr[0] += 1
                        tiles.append(g)
                return ensure, tiles

            def msg_pass(s, gather, h, accsrc, acc, last, sink, interleave=None):
                ensure, tiles = gather
                """One message-passing pass over all blocks for segment s."""
                covs = meta.covs[s]
                ncov_s = len(covs)
                colbase = covs[0][2]
                # lazily-built one-hot batches (8 coverings each)
                oh_tiles = [None] * ((ncov_s + 7) // 8)

                def get_oh(j):
                    bi = j // 8
                    if oh_tiles[bi] is None:
                        j0 = bi * 8
                        nb = min(8, ncov_s - j0)
                        oh = ohp.tile([128, 8, 128], BF16, tag="oh")
                        dcol = dstsb[:, colbase + j0:colbase + j0 + nb]
                        nc.vector.tensor_tensor(
                            oh[:, 0:nb, :], iotasb[:, 0:nb, :],
                            dcol.broadcast_to([128, nb, 128]),
                            op=OP.is_equal)
                        oh_tiles[bi] = oh
                    return oh_tiles[bi][:, j % 8, :]

                ji = 0
                for p in range(BPC):
                    # emit gather calls covering this block (plus 2 prefetch)
                    je = ji
                    while je < ncov_s and covs[je][0] == p:
                        je += 1
                    hi = covs[je - 1][1] if je > ji else (covs[ji][1] if ji < ncov_s else 0)
                    ensure(min(hi + 32, meta.nch[s] - 1))
                    psf = pp.tile([128, 256], FP32, tag="ps", name="psf")
                    ps = psf[:, 0:h]
                    has_covs = ji < ncov_s and covs[ji][0] == p
                    # inject running sum (or self-loop term for pass 0)
                    nc.tensor.matmul(ps[:], identsb[:], accsrc[:, p, :],
                                     start=True, stop=not has_covs)
                    while ji < ncov_s and covs[ji][0] == p:
                        (_, ch, _col) = covs[ji]
                        mt = tiles[ch // 8][:, ch % 8, :]
                        nc.tensor.matmul(
                            ps[:], get_oh(ji), mt,
                            start=False,
                            stop=(ji == ncov_s - 1 or covs[ji + 1][0] != p))
                        ji += 1
                    if not last:
                        nc.scalar.copy(acc[:, p, :], ps[:])
                    else:
                        sink(p, ps)
                    if interleave is not None and p % BPS == BPS - 1:
                        interleave(p // BPS)

            # ---- layer 1 message passing ----
            def z1_sink(p, ps):
                zt = sp.tile([128, cfg.h1], BF16, tag="z1")
                nc.scalar.activation(zt[:], ps[:], AT.Prelu, bias=0.0,
                                     scale=dvsb[:, p:p + 1], alpha=cfg.neg)
                s, q = p // BPS, p % BPS
                nc.sync.dma_start(z1d[s][q * 128:(q + 1) * 128, :], zt[:])

            # dense layer 2 for one segment (interleaved into mp1 last pass)
            def dense2_seg(s2):
                with tc.tile_pool(name=f"ztp{s2}", bufs=1) as ztp:
                    z1t = ztp.tile([128, kh1, BPS * 128], BF16, tag="z1t")
                    for k in range(kh1):
                        nc.sync.dma_start_transpose(
                            out=z1t[:, k, :],
                            in_=z1d[s2][:, k * 128:(k + 1) * 128])
                    for q in range(BPS):
                        p = s2 * BPS + q
                        psd = pp.tile([128, 256], FP32, tag="ps", name="psd")
                        ps = psd[:, 0:cfg.h2]
                        for k in range(kh1):
                            nc.tensor.matmul(
                                ps[:], z1t[:, k, q * 128:(q + 1) * 128],
                                w2sb[:, k, :],
                                start=(k == 0), stop=(k == kh1 - 1))
                        nc.scalar.mul(g2own[:, p, :], ps[:], dvsb[:, p:p + 1])
                        nc.sync.dma_start(g2s[s2][q * 128:(q + 1) * 128, :],
                                          g2own[:, p, :])
                if stop != "p4":
                    nc.gpsimd.collective_compute(
                        "AllGather", OP.bypass, replica_groups=rg,
                        ins=[g2s[s2].opt()], outs=[g2f[s2].opt()])

            with tc.tile_pool(name="gp1", bufs=cfg.gbufs) as gp1:
                # hold all gather queues until the last AG1 segment lands:
                # concurrent AG+gather traffic collapses AG bandwidth ~4x,
                # so let the AGs run alone at full rate first.
                for q in range(4):
                    dummy = gp1.tile([128, 8, cfg.h1], BF16, tag="g256",
                                     name="dummy")
                    nc.gpsimd.dma_gather(
                        dummy[:, 0:1, :], g1f[S - 1][:, :], idxsb[:, 0:8],
                        128, 128, cfg.h1, queue_num=q)
                for s in range(S):
                    gath = make_gatherer(s, g1f, cfg.h1, gp1)
                    last = s == S - 1
                    msg_pass(s, gath, cfg.h1,
                             accsrc=(g1own if s == 0 else acc1), acc=acc1,
                             last=last, sink=z1_sink,
                             interleave=(dense2_seg if last and stop not in
                                         ("p3",) else None))

            if stop in ("p3", "p4", "ag2"):
                nc.compile()
                return nc

            # ---- layer 2 message passing ----
            def out_sink(p, ps):
                ot = sp.tile([128, cfg.h2], FP32, tag="zo")
                nc.scalar.activation(ot[:], ps[:], AT.Prelu, bias=0.0,
                                     scale=dvsb[:, p:p + 1], alpha=cfg.neg)
                nc.sync.dma_start(out[p * 128:(p + 1) * 128, :], ot[:])

            with tc.tile_pool(name="gp2", bufs=cfg.gbufs) as gp2:
                for q in range(4):
                    dummy2 = gp2.tile([128, 8, cfg.h2], BF16, tag="g128",
                                      name="dummy2")
                    nc.gpsimd.dma_gather(
                        dummy2[:, 0:1, :], g2f[S - 1][:, :], idxsb[:, 0:8],
                        128, 128, cfg.h2, queue_num=q)
                for s in range(S):
                    gath = make_gatherer(s, g2f, cfg.h2, gp2)
                    msg_pass(s, gath, cfg.h2,
                             accsrc=(g2own if s == 0 else acc2), acc=acc2,
                             last=(s == S - 1), sink=out_sink)

    nc.compile()
    return nc


def install_ntff_hook():
    """The agent image's antenv lacks axon_hooks; graft it so trace=True
    can reach the libaxon_pjrt NTFF profiling C ABI."""
    import sys as _sys, types as _types
    if "antenv.axon_hooks" in _sys.modules:
        return
    _sys.path.insert(0, "/root/.axon_site")
    from trn_agent_boot.trn_boot import _ntff_profile_via_ctypes
    hook = _ntff_profile_via_ctypes("/opt/axon/libaxon_pjrt.so")
    mod = _types.ModuleType("antenv.axon_hooks")
    mod._hook = hook
    mod.get_axon_ntff_profile_hook = lambda: mod._hook
    mod.set_axon_ntff_profile_hook = lambda h: setattr(mod, "_hook", h)
    _sys.modules["antenv.axon_hooks"] = mod
    import antenv
    antenv.axon_hooks = mod


def run(cfg: Cfg, X, edge_index, W1, b1, W2, b2, trace=False,
        stop_after='full', trace_cores=None):
    if trace:
        install_ntff_hook()
    import time
    t0 = time.time()
    in_maps, meta = preprocess(cfg, X, edge_index, W1, b1, W2, b2)
    t1 = time.time()
    nc = build(cfg, meta, stop_after=stop_after)
    t2 = time.time()
    print(f"preprocess {t1-t0:.1f}s, build+compile {t2-t1:.1f}s", flush=True)
    res = run_bass_kernel_spmd(nc, in_maps, core_ids=list(range(cfg.cores)),
                               trace=trace, trace_cores=trace_cores)
    print(f"hw run {time.time()-t2:.1f}s", flush=True)
    nblk = cfg.npad // 128
    full = np.empty((cfg.npad, cfg.h2), np.float32)
    for c in range(cfg.cores):
        o = res.results[c]["out"]
        for p, b in enumerate(range(c, nblk, cfg.cores)):
            full[b * 128:(b + 1) * 128] = o[p * 128:(p + 1) * 128]
    full = full[:cfg.n]
    return full, res, nc, in_maps, meta


_CFG = Cfg(n=50000, e=800000, d_in=512, h1=256, h2=128, cores=8)


def kernel(X, edge_index, W1, b1, W2, b2):
    full, _res, _nc, _maps, _meta = run(
        _CFG, X, edge_index, W1, b1, W2, b2, trace=False)
    return full


# revision 15
# speedup vs baseline: 1.1257x; 1.1257x over previous
"""Self-contained Trainium2 Bass kernel for a 2-layer GCN encoder
(PyG GCNConv x2 with LeakyReLU), distributed over 8 NeuronCores.

kernel(**inputs) takes the full unsharded inputs (X [50000,512] f32,
edge_index [2,800000] int64, W1/b1/W2/b2) and returns the full
[50000,128] f32 output. See build() for the device program.
"""

import sys
if "/opt/trn_rl_repo" not in sys.path:
    sys.path.insert(0, "/opt/trn_rl_repo")

import math
from dataclasses import dataclass, field

import numpy as np
import ml_dtypes

import concourse.bass as bass
import concourse.tile as tile
from concourse import bacc, mybir
from concourse.bass_utils import run_bass_kernel_spmd

FP32 = mybir.dt.float32
BF16 = mybir.dt.bfloat16
I32 = mybir.dt.int32
I16 = mybir.dt.int16


@dataclass
class Cfg:
    n: int          # real node count
    e: int          # real edge count
    d_in: int
    h1: int
    h2: int
    cores: int = 8
    bpc: int = 50   # 128-node dst blocks per core
    split: int = 32768   # int16 gather index boundary
    grp: int = 4    # dst blocks per dma_gather call
    neg: float = 0.2

    @property
    def npad(self):
        return self.cores * self.bpc * 128

    @property
    def shard(self):
        return self.bpc * 128


@dataclass
class Meta:
    cpa: int  # chunks per block, side A (src < split)
    cpb: int  # chunks per block, side B
    bias1_nz: bool
    bias2_nz: bool
    # per-group idx-tile column offsets: list of (g0, gn, colA, colB)
    groups: list = field(default_factory=list)
    tot_cols: int = 0   # idx tile columns (int16 packed by 16)
    nch: int = 0        # chunks per block total


def preprocess(cfg: Cfg, X, edge_index, W1, b1, W2, b2):
    """Host-side: shard + edge partitioning. Returns (in_maps, meta)."""
    n, npad, shard = cfg.n, cfg.npad, cfg.shard
    src = np.asarray(edge_index[0], dtype=np.int64)
    dst = np.asarray(edge_index[1], dtype=np.int64)

    deg = np.bincount(dst, minlength=npad).astype(np.float32) + 1.0
    dinv = (1.0 / np.sqrt(deg)).astype(np.float32)

    # self loops for every (padded) node
    asrc = np.concatenate([src, np.arange(npad, dtype=np.int64)])
    adst = np.concatenate([dst, np.arange(npad, dtype=np.int64)])

    # dst blocks are assigned round-robin to cores, and each core's shard is
    # split into two position-halves that are allgathered separately (so the
    # second collective can overlap message passing). The g tables hold node
    # n at row perm[n] of table half[n]; gather indices are half-relative.
    hb = cfg.bpc // 2
    nb_ = np.arange(npad, dtype=np.int64) >> 7
    c_ = nb_ % cfg.cores
    p_ = nb_ // cfg.cores
    half_ = (p_ >= hb).astype(np.int64)
    perm = (half_ * (npad // 2) + c_ * (hb * 128) + (p_ - half_ * hb) * 128
            + (np.arange(npad, dtype=np.int64) & 127))
    asrc = perm[asrc]

    blk = adst >> 7
    side = (asrc >= npad // 2).astype(np.int64)
    order = np.lexsort((asrc, side, blk))
    asrc, adst, blk, side = asrc[order], adst[order], blk[order], side[order]

    nblk = npad // 128
    cnt_a = np.bincount(blk[side == 0], minlength=nblk)
    cnt_b = np.bincount(blk[side == 1], minlength=nblk)
    cpa = int(math.ceil(cnt_a.max() / 128)) if cnt_a.max() > 0 else 0
    cpb = int(math.ceil(cnt_b.max() / 128)) if cnt_b.max() > 0 else 0
    cap_a, cap_b = cpa * 128, cpb * 128
    nch = cpa + cpb

    nrows_a = npad // 2
    nrows_b = npad // 2
    spread = (np.arange(max(cap_a, cap_b, 1), dtype=np.int64) * 67)
    idx_a = ((spread[:cap_a] + 97) % nrows_a).astype(np.int16)[None, :] \
        * np.ones((nblk, 1), np.int16) if cap_a else np.zeros((nblk, 1), np.int16)
    idx_a = np.ascontiguousarray(
        ((np.arange(nblk)[:, None] * 997 + spread[None, :cap_a]) % nrows_a
         ).astype(np.int16)) if cap_a else np.zeros((nblk, 1), np.int16)
    idx_b = np.ascontiguousarray(
        ((np.arange(nblk)[:, None] * 997 + spread[None, :cap_b]) % nrows_b
         ).astype(np.int16)) if cap_b else np.zeros((nblk, 1), np.int16)
    assert cfg.bpc % 2 == 0
    dstloc = np.full((nblk, nch * 128), -1, np.int32)

    mask = side == 0
    b_, s_, d_ = blk[mask], asrc[mask], adst[mask]
    start = np.zeros(nblk + 1, np.int64)
    np.cumsum(cnt_a, out=start[1:])
    pos = np.arange(len(b_)) - start[b_]
    idx_a[b_, pos] = s_.astype(np.int16)
    dstloc[b_, pos] = (d_ & 127).astype(np.int32)

    mask = side == 1
    b_, s_, d_ = blk[mask], asrc[mask], adst[mask]
    start = np.zeros(nblk + 1, np.int64)
    np.cumsum(cnt_b, out=start[1:])
    pos = np.arange(len(b_)) - start[b_]
    idx_b[b_, pos] = (s_ - npad // 2).astype(np.int16)
    dstloc[b_, cap_a + pos] = (d_ & 127).astype(np.int32)

    # group layout for gather calls (identical structure on every core)
    groups = []
    col = 0
    for g0 in range(0, cfg.bpc, cfg.grp):
        gn = min(cfg.grp, cfg.bpc - g0)
        col_a = col
        col_b = col + gn * cap_a // 16
        col = col_b + gn * cap_b // 16
        groups.append((g0, gn, col_a, col_b))
    tot_cols = col

    meta = Meta(
        cpa=cpa, cpb=cpb,
        bias1_nz=bool(np.any(np.asarray(b1) != 0)),
        bias2_nz=bool(np.any(np.asarray(b2) != 0)),
        groups=groups, tot_cols=tot_cols, nch=nch,
    )

    # replicated tensors
    XT = np.zeros((cfg.d_in, npad), np.float32)
    XT[:, :n] = np.asarray(X, np.float32).T
    XT = XT.astype(ml_dtypes.bfloat16)
    W1b = np.asarray(W1, np.float32).astype(ml_dtypes.bfloat16)
    W2b = np.asarray(W2, np.float32).astype(ml_dtypes.bfloat16)
    iota4 = np.ascontiguousarray(np.broadcast_to(
        np.arange(128, dtype=np.float32)[None, None, :],
        (128, 8, 128))).astype(ml_dtypes.bfloat16)
    dinv_full = np.ascontiguousarray(
        dinv.reshape(nblk, 128).T).astype(np.float32)   # [128, nblk]

    in_maps = []
    for c in range(cfg.cores):
        blocks = list(range(c, nblk, cfg.cores))   # round-robin assignment
        # flat int16 idx stream in group order: [A segs of group][B segs]
        parts = []
        for (g0, gn, _ca, _cb) in groups:
            bsel = blocks[g0:g0 + gn]
            parts.append(idx_a[bsel, :cap_a].reshape(-1))
            parts.append(idx_b[bsel, :cap_b].reshape(-1))
        flat = np.concatenate(parts) if parts else np.zeros(0, np.int16)
        assert flat.size == tot_cols * 16, (flat.size, tot_cols * 16)
        idx_tile = np.ascontiguousarray(
            np.tile(flat.reshape(-1, 16).T, (8, 1)))          # [128, tot_cols]

        dst_tile = np.ascontiguousarray(
            dstloc[blocks].reshape(cfg.bpc * nch, 128).T)      # [128, bpc*nch]

        node_sel = (np.asarray(blocks)[:, None] * 128
                    + np.arange(128)[None, :]).reshape(-1)
        dv = dinv[node_sel].reshape(cfg.bpc, 128).T
        m = {
            "xt": np.ascontiguousarray(XT[:, node_sel]),
            "w1": W1b, "w2": W2b,
            "idx": idx_tile,
            "dstloc": dst_tile.astype(np.float32).astype(ml_dtypes.bfloat16),
            "dinv": np.ascontiguousarray(dv).astype(np.float32),
            "dinv08": np.ascontiguousarray(dv * (1.0 - cfg.neg)).astype(np.float32),
            "dinv02": np.ascontiguousarray(dv * cfg.neg).astype(np.float32),
            "iota4": iota4,
        }
        if meta.bias1_nz:
            m["b1bc"] = np.ascontiguousarray(np.broadcast_to(
                np.asarray(b1, np.float32)[None, :], (128, cfg.h1))).astype(np.float32)
        if meta.bias2_nz:
            m["b2bc"] = np.ascontiguousarray(np.broadcast_to(
                np.asarray(b2, np.float32)[None, :], (128, cfg.h2))).astype(np.float32)
        in_maps.append(m)
    return in_maps, meta


def build(cfg: Cfg, meta: Meta, stop_after: str = 'full'):
    nc = bacc.Bacc("TRN2", target_bir_lowering=False, debug=False,
                   num_devices=cfg.cores, num_swdge_queues=4)
    sh, npad = cfg.shard, cfg.npad
    kin, kh1 = cfg.d_in // 128, cfg.h1 // 128
    cpa, cpb, nch = meta.cpa, meta.cpb, meta.nch
    cap_a, cap_b = cpa * 128, cpb * 128
    nrows_h = npad // 2
    hb = cfg.bpc // 2
    AT = mybir.ActivationFunctionType
    OP = mybir.AluOpType

    xt = nc.dram_tensor("xt", [cfg.d_in, sh], BF16, kind="ExternalInput")
    w1 = nc.dram_tensor("w1", [cfg.d_in, cfg.h1], BF16, kind="ExternalInput")
    w2 = nc.dram_tensor("w2", [cfg.h1, cfg.h2], BF16, kind="ExternalInput")
    idx = nc.dram_tensor("idx", [128, meta.tot_cols], I16, kind="ExternalInput")
    dstloc = nc.dram_tensor("dstloc", [128, cfg.bpc * nch], BF16, kind="ExternalInput")
    dinv = nc.dram_tensor("dinv", [128, cfg.bpc], FP32, kind="ExternalInput")
    dinv08 = nc.dram_tensor("dinv08", [128, cfg.bpc], FP32, kind="ExternalInput")
    dinv02 = nc.dram_tensor("dinv02", [128, cfg.bpc], FP32, kind="ExternalInput")
    iota_d = nc.dram_tensor("iota4", [128, 8, 128], BF16, kind="ExternalInput")
    b1bc = (nc.dram_tensor("b1bc", [128, cfg.h1], FP32, kind="ExternalInput")
            if meta.bias1_nz else None)
    b2bc = (nc.dram_tensor("b2bc", [128, cfg.h2], FP32, kind="ExternalInput")
            if meta.bias2_nz else None)
    out = nc.dram_tensor("out", [sh, cfg.h2], FP32, kind="ExternalOutput")

    rg = [list(range(cfg.cores))]
    stop = stop_after

    with tile.TileContext(nc) as tc:
        with (
            tc.tile_pool(name="constp", bufs=1) as constp,
            tc.tile_pool(name="dram", bufs=1, space="DRAM") as dram,
            tc.tile_pool(name="ohp", bufs=8) as ohp,
            tc.tile_pool(name="sp", bufs=4) as sp,
            tc.tile_pool(name="pp", bufs=6, space="PSUM") as pp,
        ):
            g1s0 = dram.tile([sh // 2, cfg.h1], BF16)
            g1s1 = dram.tile([sh // 2, cfg.h1], BF16)
            g1f0 = dram.tile([nrows_h, cfg.h1], BF16)
            g1f1 = dram.tile([nrows_h, cfg.h1], BF16)
            z1d0 = dram.tile([sh // 2, cfg.h1], BF16)
            z1d1 = dram.tile([sh // 2, cfg.h1], BF16)
            g2s0 = dram.tile([sh // 2, cfg.h2], BF16)
            g2s1 = dram.tile([sh // 2, cfg.h2], BF16)
            g2f0 = dram.tile([nrows_h, cfg.h2], BF16)
            g2f1 = dram.tile([nrows_h, cfg.h2], BF16)

            # ---- constants ----
            w1sb = constp.tile([128, kin, cfg.h1], BF16)
            for k in range(kin):
                nc.sync.dma_start(w1sb[:, k, :], w1[k * 128:(k + 1) * 128, :])
            w2sb = constp.tile([128, kh1, cfg.h2], BF16)
            for k in range(kh1):
                nc.sync.dma_start(w2sb[:, k, :], w2[k * 128:(k + 1) * 128, :])
            idxsb = constp.tile([128, meta.tot_cols], I16)
            nc.sync.dma_start(idxsb[:], idx[:])
            dstsb = constp.tile([128, cfg.bpc * nch], BF16)
            nc.sync.dma_start(dstsb[:], dstloc[:])
            dvsb = constp.tile([128, cfg.bpc], FP32)
            nc.sync.dma_start(dvsb[:], dinv[:])
            d08sb = constp.tile([128, cfg.bpc], FP32)
            nc.sync.dma_start(d08sb[:], dinv08[:])
            d02sb = constp.tile([128, cfg.bpc], FP32)
            nc.sync.dma_start(d02sb[:], dinv02[:])
            iotasb = constp.tile([128, 8, 128], BF16)
            nc.sync.dma_start(iotasb[:], iota_d[:])
            b1sb = b2sb = None
            if b1bc is not None:
                b1sb = constp.tile([128, cfg.h1], FP32)
                nc.sync.dma_start(b1sb[:], b1bc[:])
            if b2bc is not None:
                b2sb = constp.tile([128, cfg.h2], FP32)
                nc.sync.dma_start(b2sb[:], b2bc[:])

            # ---- dense phase helper: g = dinv * (inT-tiles @ W) ----
            # processes position-halves [b0, b1); insb columns are relative
            def dense(insb, wsb, kk, h, sink, b0, b1):
                for b in range(b0, b1):
                    rb = b - b0
                    ps = pp.tile([128, h], FP32, tag="ps")
                    for k in range(kk):
                        nc.tensor.matmul(ps[:], insb[:, k, rb * 128:(rb + 1) * 128],
                                         wsb[:, k, :],
                                         start=(k == 0), stop=(k == kk - 1))
                    gt = sp.tile([128, h], BF16, tag="gt")
                    nc.scalar.mul(gt[:], ps[:], dvsb[:, b:b + 1])
                    nc.sync.dma_start(sink[rb * 128:(rb + 1) * 128, :], gt[:])

            qctr = [0, 0]

            def gather_seg(dst_tile, src_ap, nchunks, col0, h, side):
                """dma_gather calls of <=8 chunks (1024 idx HW limit).
                Side A uses queues 0/1, side B queues 2/3, so B calls
                waiting on the second allgather never stall A calls."""
                for off in range(0, nchunks, 8):
                    nck = min(8, nchunks - off)
                    nc.gpsimd.dma_gather(
                        dst_tile[:, off:off + nck, :], src_ap,
                        idxsb[:, col0 + off * 8: col0 + (off + nck) * 8],
                        nck * 128, nck * 128, h,
                        queue_num=side * 2 + qctr[side] % 2)
                    qctr[side] += 1

            def message_pass(gpa, gpb, gla, glb, h, z_sink, bsb, only=None,
                             grange=None):
                groups = (meta.groups if only is None else meta.groups[:only])
                if grange is not None:
                    groups = meta.groups[grange[0]:grange[1]]
                for (g0, gn, col_a, col_b) in groups:
                    ga = gb = None
                    if cpa:
                        ga = gpa.tile([128, cfg.grp * cpa, h], BF16, tag="gA")
                        gather_seg(ga, gla, gn * cpa, col_a, h, 0)
                    if cpb:
                        gb = gpb.tile([128, cfg.grp * cpb, h], BF16, tag="gB")
                        gather_seg(gb, glb, gn * cpb, col_b, h, 1)
                    for j in range(gn):
                        b = g0 + j
                        ps = pp.tile([128, h], FP32, tag="ps")
                        for c0 in range(0, nch, 8):
                            nb = min(8, nch - c0)
                            oh = ohp.tile([128, 8, 128], BF16, tag="oh")
                            dcol = dstsb[:, b * nch + c0:b * nch + c0 + nb]
                            nc.vector.tensor_tensor(
                                oh[:, 0:nb, :], iotasb[:, 0:nb, :],
                                dcol.broadcast_to([128, nb, 128]),
                                op=OP.is_equal)
                            for c in range(c0, c0 + nb):
                                if c < cpa:
                                    mt = ga[:, j * cpa + c, :]
                                else:
                                    mt = gb[:, j * cpb + (c - cpa), :]
                                nc.tensor.matmul(ps[:], oh[:, c - c0, :], mt,
                                                 start=(c == 0),
                                                 stop=(c == nch - 1))
                        if bsb is None:
                            r = sp.tile([128, h], FP32, tag="r")
                            nc.scalar.activation(r[:], ps[:], AT.Relu,
                                                 bias=0.0, scale=d08sb[:, b:b + 1])
                            z_sink(b, ps, d02sb[:, b:b + 1], r)
                        else:
                            t = sp.tile([128, h], FP32, tag="t")
                            nc.vector.tensor_scalar(t[:], ps[:], dvsb[:, b:b + 1],
                                                    None, op0=OP.mult)
                            t2 = sp.tile([128, h], FP32, tag="t2")
                            nc.vector.tensor_tensor(t2[:], t[:], bsb[:], op=OP.add)
                            r = sp.tile([128, h], FP32, tag="r")
                            nc.scalar.activation(r[:], t2[:], AT.Relu,
                                                 bias=0.0, scale=1.0 - cfg.neg)
                            z_sink(b, t2, cfg.neg, r)

            def z1_sink(b, acc, coef, r):
                z = sp.tile([128, cfg.h1], BF16, tag="z1")
                nc.vector.scalar_tensor_tensor(z[:], acc[:], coef, r[:],
                                               op0=OP.mult, op1=OP.add)
                zt, rb = (z1d0, b) if b < hb else (z1d1, b - hb)
                nc.sync.dma_start(zt[rb * 128:(rb + 1) * 128, :], z[:])

            def out_sink(b, acc, coef, r):
                z = sp.tile([128, cfg.h2], FP32, tag="zo")
                nc.vector.scalar_tensor_tensor(z[:], acc[:], coef, r[:],
                                               op0=OP.mult, op1=OP.add)
                nc.sync.dma_start(out[b * 128:(b + 1) * 128, :], z[:])

            # ---- phase 1: g1 shard (two halves, allgathered separately) ----
            with tc.tile_pool(name="xtp", bufs=1) as xtp:
                xt0 = xtp.tile([128, kin, hb * 128], BF16)
                xt1 = xtp.tile([128, kin, sh - hb * 128], BF16)
                for k in range(kin):
                    nc.sync.dma_start(xt0[:, k, :],
                                      xt[k * 128:(k + 1) * 128, 0:hb * 128])
                    nc.sync.dma_start(xt1[:, k, :],
                                      xt[k * 128:(k + 1) * 128, hb * 128:])
                dense(xt0, w1sb, kin, cfg.h1, g1s0, 0, hb)
                if stop != "p1":
                    nc.gpsimd.collective_compute(
                        "AllGather", OP.bypass, replica_groups=rg,
                        ins=[g1s0.opt()], outs=[g1f0.opt()])
                dense(xt1, w1sb, kin, cfg.h1, g1s1, hb, cfg.bpc)
                if stop != "p1":
                    nc.gpsimd.collective_compute(
                        "AllGather", OP.bypass, replica_groups=rg,
                        ins=[g1s1.opt()], outs=[g1f1.opt()])

            # ---- phases 3+4 interleaved: layer-1 message passing with
            # dense2+AG2 for each z1 half emitted as soon as that half's
            # blocks are sunk, so AG2 overlaps mp1's second half instead of
            # running in a dead window after it.
            if stop not in ("p1", "ag1"):
                only = 1 if stop == "p3one" else None
                with tc.tile_pool(name="gp1a", bufs=3) as gp1a, \
                        tc.tile_pool(name="gp1b", bufs=3) as gp1b, \
                        tc.tile_pool(name="ztp", bufs=2) as ztp:

                    def dense2_half(half, zt, gs, gf):
                        z1t = ztp.tile([128, kh1, sh // 2], BF16, tag="z1t",
                                       name="z1t")
                        for k in range(kh1):
                            nc.sync.dma_start_transpose(
                                out=z1t[:, k, :],
                                in_=zt[:, k * 128:(k + 1) * 128])
                        dense(z1t, w2sb, kh1, cfg.h2, gs,
                              half * hb, half * hb + hb)
                        if stop not in ("p4",):
                            nc.gpsimd.collective_compute(
                                "AllGather", OP.bypass, replica_groups=rg,
                                ins=[gs.opt()], outs=[gf.opt()])

                    if only is not None or stop in ("p3",):
                        message_pass(gp1a, gp1b, g1f0[:, :], g1f1[:, :],
                                     cfg.h1, z1_sink, b1sb, only=only)
                    else:
                        ngrp = len(meta.groups)
                        cut = (hb + cfg.grp - 1) // cfg.grp
                        message_pass(gp1a, gp1b, g1f0[:, :], g1f1[:, :],
                                     cfg.h1, z1_sink, b1sb, grange=(0, cut))
                        dense2_half(0, z1d0, g2s0, g2f0)
                        message_pass(gp1a, gp1b, g1f0[:, :], g1f1[:, :],
                                     cfg.h1, z1_sink, b1sb,
                                     grange=(cut, ngrp))
                        dense2_half(1, z1d1, g2s1, g2f1)

            if stop not in ("p1", "ag1", "p3", "p3one", "p4"):
                # ---- phase 6: layer-2 message passing -> out (f32) ----
                with tc.tile_pool(name="gp2a", bufs=3) as gp2a, \
                        tc.tile_pool(name="gp2b", bufs=3) as gp2b:
                    message_pass(gp2a, gp2b, g2f0[:, :], g2f1[:, :], cfg.h2,
                                 out_sink, b2sb)

    nc.compile()
    return nc


def install_ntff_hook():
    """The agent image's antenv lacks axon_hooks; graft it so trace=True
    can reach the libaxon_pjrt NTFF profiling C ABI."""
    import sys as _sys, types as _types
    if "antenv.axon_hooks" in _sys.modules:
        return
    _sys.path.insert(0, "/root/.axon_site")
    from trn_agent_boot.trn_boot import _ntff_profile_via_ctypes
    hook = _ntff_profile_via_ctypes("/opt/axon/libaxon_pjrt.so")
    mod = _types.ModuleType("antenv.axon_hooks")
    mod._hook = hook
    mod.get_axon_ntff_profile_hook = lambda: mod._hook
    mod.set_axon_ntff_profile_hook = lambda h: setattr(mod, "_hook", h)
    _sys.modules["antenv.axon_hooks"] = mod
    import antenv
    antenv.axon_hooks = mod


def run(cfg: Cfg, X, edge_index, W1, b1, W2, b2, trace=False,
        stop_after='full', trace_cores=None):
    if trace:
        install_ntff_hook()
    import time
    t0 = time.time()
    in_maps, meta = preprocess(cfg, X, edge_index, W1, b1, W2, b2)
    t1 = time.time()
    nc = build(cfg, meta, stop_after=stop_after)
    t2 = time.time()
    print(f"preprocess {t1-t0:.1f}s, build+compile {t2-t1:.1f}s", flush=True)
    res = run_bass_kernel_spmd(nc, in_maps, core_ids=list(range(cfg.cores)),
                               trace=trace, trace_cores=trace_cores)
    print(f"hw run {time.time()-t2:.1f}s", flush=True)
    nblk = cfg.npad // 128
    full = np.empty((cfg.npad, cfg.h2), np.float32)
    for c in range(cfg.cores):
        o = res.results[c]["out"]
        for p, b in enumerate(range(c, nblk, cfg.cores)):
            full[b * 128:(b + 1) * 128] = o[p * 128:(p + 1) * 128]
    full = full[:cfg.n]
    return full, res, nc, in_maps, meta


_CFG = Cfg(n=50000, e=800000, d_in=512, h1=256, h2=128,
           cores=8, bpc=50, split=32768, grp=2)


def kernel(X, edge_index, W1, b1, W2, b2):
    full, _res, _nc, _maps, _meta = run(
        _CFG, X, edge_index, W1, b1, W2, b2, trace=False)
    return full

